# revision 11
# baseline (speedup 1.0000x reference)
"""DiscreteBipartiteFlow forward on 8 trn2 NeuronCores.

Math: inputs rows are exact one-hots (x0|x1). net = relu(x0@W1+b1)@W2+b2
only depends on i0=argmax(x0), so precompute (on device, per core) the
[V, 2V] table NET = relu(W1+b1)@W2+b2 and its per-row argmaxes
L[i]=argmax(NET[i,:V]), S[i]=argmax(NET[i,V:]). The straight-through
one_hot_argmax is numerically exactly-hard, one_hot_multiply of a
one-hot x1 by the one-hot scale is an index product, and one_hot_add is
an index sum, so z1 = one_hot((L[i0] + a1*S[i0]) mod V) (or 0 when
S[i0]==0, since scale index 0 is excluded). Output = [x0 | z1].

Per-core structure (1024 rows, 8 rows per partition):
 - loads on the SP queue, stores on the ACT queue (separate DMA rings);
   weights split w1t+b1 / w2(k01) / w2(k23)+b2 so relu (ACT engine,
   fused bias+relu) and the fp32 NET matmul start as early as possible.
   S-head matmuls run before L-head so the scale argmax + its derived
   scalars hide under the L-head matmuls.
 - a1 = x1 . iota via per-slot scalar_tensor_tensor dots (accum_out)
   on Vector while the PE runs NET; comb = pack[i0] via the same dot
   trick against packB (pack table replicated along free axis by one
   PE matmul: lhsT = pk broadcast per-partition, rhs = identity), split
   Vector/GpSimd. pack = S + 128*L + 16384*[S==0], all < 2^24 so fp32
   dots are exact.
 - int unpack (proven ts ops + STT with [P,1] AP scalars), then
   z1 = is_equal(iota, c) per 2-slot chunk, alternating Vector/GpSimd,
   each chunk streaming out on the ACT queue as it finishes.
 - iota/identity built on device (gpsimd.iota); x0 passthrough written
   straight from the loaded x tile.
Data-parallel over 8 cores; weights replicated (host marshalling only).
"""

import numpy as np

V = 128
H = 512
N_CORES = 8
P = 128
NJ = 8               # row slots per partition
KH = H // P          # 4 contraction chunks

W1T_OFF = 0          # [P, 512]  w1t[p, k*128+i] = W1[i, 128k+p]
B1_OFF = 512         # [P, 4]    b1[p, k] = b1[128k+p]
W2_OFF = 516         # [P, 1024] w2[p, k*256+c] = W2[128k+p, c]
B2_OFF = 1540        # [P, 256]  b2 replicated per partition
WB_COLS = 1796


def build_bass(rows: int):
    """Build the single-core Bass program for a [rows, 2V] batch shard."""
    import concourse.bacc as bacc
    import concourse.bass as bass
    import concourse.tile as tile
    from concourse import mybir

    f32 = mybir.dt.float32
    i32 = mybir.dt.int32
    u32 = mybir.dt.uint32
    A = mybir.AluOpType
    AF = mybir.ActivationFunctionType

    assert rows == P * NJ

    nc = bacc.Bacc(None)
    x = nc.declare_dram_parameter("x", [rows, 2 * V], f32, isOutput=False)
    wb = nc.declare_dram_parameter("wb", [P, WB_COLS], f32, isOutput=False)
    out = nc.declare_dram_parameter("out", [rows, 2 * V], f32, isOutput=True)

    x_r = x.rearrange("(p j) n -> p j n", j=NJ)
    out_r = out.rearrange("(p j) n -> p j n", j=NJ)

    def bcast_mid(t_ap, reps):
        return bass.AP(
            tensor=t_ap.tensor, offset=t_ap.offset,
            ap=[t_ap.ap[0], [0, reps]] + list(t_ap.ap[1:]),
        )

    def bcast_last(t_ap, reps):
        return bass.AP(
            tensor=t_ap.tensor, offset=t_ap.offset,
            ap=list(t_ap.ap) + [[0, reps]],
        )

    with tile.TileContext(nc) as tc:
        with (
            tc.tile_pool(name="main", bufs=1) as main,
            tc.tile_pool(name="psum_net", bufs=1, space="PSUM") as psum_net,
            tc.tile_pool(name="psum_pb", bufs=1, space="PSUM") as psum_pb,
        ):
            # ---- input DMAs: all on the SP queue ring, weights first ----
            wb_sb = main.tile([P, WB_COLS], f32)
            nc.sync.dma_start(out=wb_sb[:, 0:B2_OFF - 1024], in_=wb[:, 0:B2_OFF - 1024])          # w1t+b1
            nc.sync.dma_start(out=wb_sb[:, 516:1028], in_=wb[:, 516:1028])      # w2 k0,k1
            nc.sync.dma_start(out=wb_sb[:, 1028:WB_COLS], in_=wb[:, 1028:WB_COLS])  # w2 k2,k3 + b2
            xw = main.tile([P, NJ, 2 * V], f32)
            nc.sync.dma_start(out=xw[:, 0:4, :], in_=x_r[:, 0:4, :])
            nc.sync.dma_start(out=xw[:, 4:8, :], in_=x_r[:, 4:8, :])

            w1t = wb_sb[:, W1T_OFF:W1T_OFF + KH * V].rearrange("p (k i) -> p k i", k=KH)
            b1s = wb_sb[:, B1_OFF:B1_OFF + KH]
            w2 = wb_sb[:, W2_OFF:W2_OFF + KH * 2 * V].rearrange("p (k c) -> p k c", k=KH)
            b2B = wb_sb[:, B2_OFF:B2_OFF + 2 * V]

            # ---- on-device constants (GpSimd/Vector, hidden under DMAs) ----
            iota_f = main.tile([P, V], f32)
            nc.gpsimd.iota(iota_f, pattern=[[1, V]], base=0, channel_multiplier=0,
                           allow_small_or_imprecise_dtypes=True)
            ipart_f = main.tile([P, 1], f32)
            nc.gpsimd.iota(ipart_f, pattern=[[0, 1]], base=0, channel_multiplier=1,
                           allow_small_or_imprecise_dtypes=True)
            ident = main.tile([P, V], f32)
            nc.vector.tensor_tensor(out=ident, in0=iota_f, in1=bcast_last(ipart_f, V),
                                    op=A.is_equal)

            # ---- a1 dots on Vector while PE runs the table matmul ----
            a1f = main.tile([P, NJ], f32)
            dot_scr = main.tile([P, NJ, V], f32)
            for j in range(NJ):
                nc.vector.scalar_tensor_tensor(
                    out=dot_scr[:, j, :], in0=xw[:, j, V:2 * V], scalar=1.0,
                    in1=iota_f, op0=A.bypass, op1=A.mult,
                    accum_out=a1f[:, j:j + 1],
                )
            a1i = main.tile([P, NJ], i32)
            nc.gpsimd.tensor_copy(a1i, a1f)

            # ---- x0 passthrough (ACT queue ring) ----
            nc.scalar.dma_start(out=out_r[:, 0:4, 0:V], in_=xw[:, 0:4, 0:V])
            nc.scalar.dma_start(out=out_r[:, 4:8, 0:V], in_=xw[:, 4:8, 0:V])

            # ---- table: h = relu(w1t + b1) on ACT; NET = h @ W2 (+b2) on PE ----
            hT = main.tile([P, KH, V], f32)
            for k in range(KH):
                nc.scalar.activation(out=hT[:, k, :], in_=w1t[:, k, :], func=AF.Relu,
                                     bias=b1s[:, k:k + 1], scale=1.0)
            net_ps = psum_net.tile([P, 2 * V], f32)
            # S-head (cols V:2V) first so its argmax hides under L-head MMs
            for head in (1, 0):
                for k in range(KH):
                    nc.tensor.matmul(
                        net_ps[:, head * V:(head + 1) * V], lhsT=hT[:, k, :],
                        rhs=w2[:, k, head * V:(head + 1) * V],
                        start=(k == 0), stop=(k == KH - 1),
                    )

            # per head: PSUM->SBUF on ACT, b2-add on GpSimd, argmax on Vector
            # (GpSimd cannot touch PSUM)
            net_cp = main.tile([P, 2 * V], f32)
            net_sb = main.tile([P, 2 * V], f32)
            idx = []
            for head in (1, 0):
                nc.scalar.copy(net_cp[:, head * V:(head + 1) * V],
                               net_ps[:, head * V:(head + 1) * V])
                nc.gpsimd.tensor_add(net_sb[:, head * V:(head + 1) * V],
                                     net_cp[:, head * V:(head + 1) * V],
                                     b2B[:, head * V:(head + 1) * V])
                m8 = main.tile([P, 8], f32, tag=f"m8{head}")
                nc.vector.max(m8, net_sb[:, head * V:(head + 1) * V])
                ix = main.tile([P, 8], u32, tag=f"ix{head}")
                nc.vector.max_index(ix, m8, net_sb[:, head * V:(head + 1) * V])
                idx.append(ix)
            idxS, idxL = idx

            # scale-side scalars (early, hidden under L-head MMs)
            sf = main.tile([P, 1], f32)
            nc.vector.tensor_copy(sf, idxS[:, 0:1])
            zc01 = main.tile([P, 1], f32)
            nc.vector.tensor_scalar(out=zc01, in0=sf, scalar1=0.5, scalar2=None,
                                    op0=A.is_lt)
            # pack column: pk = S + 128*L + 16384*[S==0]
            lf = main.tile([P, 1], f32)
            nc.vector.tensor_copy(lf, idxL[:, 0:1])
            pk0 = main.tile([P, 1], f32)
            nc.vector.tensor_scalar(out=pk0, in0=lf, scalar1=float(V), scalar2=sf[:, 0:1],
                                    op0=A.mult, op1=A.add)
            pk = main.tile([P, 1], f32)
            nc.vector.tensor_scalar(out=pk, in0=zc01, scalar1=float(V * V),
                                    scalar2=pk0[:, 0:1], op0=A.mult, op1=A.add)
            # replicate along free axis, then PE-broadcast to packB[m, v] = pk[v]
            pk_rep = main.tile([P, V], f32)
            nc.vector.tensor_copy(pk_rep, bcast_last(pk[:, 0:1], V))
            packB = psum_pb.tile([P, V], f32)
            nc.tensor.matmul(packB, lhsT=pk_rep, rhs=ident, start=True, stop=True)
            packB_sb = main.tile([P, V], f32)
            nc.scalar.copy(packB_sb, packB)

            # ---- comb dots: comb[p,j] = sum_v x0[p,j,v] * pk[v] ----
            # All on Vector (Pool can't run TensorScalarPtr). First two read
            # packB straight from PSUM; the rest use the SBUF copy.
            combf = main.tile([P, NJ], f32)
            for j in range(NJ):
                src = packB if j < 2 else packB_sb
                nc.vector.scalar_tensor_tensor(
                    out=dot_scr[:, j, :], in0=xw[:, j, 0:V], scalar=1.0,
                    in1=src, op0=A.bypass, op1=A.mult,
                    accum_out=combf[:, j:j + 1],
                )

            # ---- unpack: c = ((L + a1*S) & 127) | 128*[S==0] ----
            # V-chain: ci -> wi -> li -> t2 -> c0 -> cc -> cf; GpSimd branch
            # computes si/sa/zi in the shadow.
            ci = main.tile([P, NJ], i32)
            nc.vector.tensor_copy(ci, combf)
            si = main.tile([P, NJ], i32)
            nc.vector.tensor_scalar(out=si, in0=ci, scalar1=V - 1, scalar2=None,
                                    op0=A.bitwise_and)
            sa = main.tile([P, NJ], i32)
            nc.gpsimd.tensor_mul(sa, si, a1i)
            wi = main.tile([P, NJ], i32)
            nc.vector.tensor_scalar(out=wi, in0=ci, scalar1=7, scalar2=None,
                                    op0=A.arith_shift_right)
            zi = main.tile([P, NJ], i32)
            nc.vector.tensor_scalar(out=zi, in0=wi, scalar1=V, scalar2=None,
                                    op0=A.bitwise_and)
            li = main.tile([P, NJ], i32)
            nc.vector.tensor_scalar(out=li, in0=wi, scalar1=V - 1, scalar2=None,
                                    op0=A.bitwise_and)
            t2 = main.tile([P, NJ], i32)
            nc.vector.tensor_add(t2, sa, li)
            c0 = main.tile([P, NJ], i32)
            nc.vector.tensor_scalar(out=c0, in0=t2, scalar1=V - 1, scalar2=None,
                                    op0=A.bitwise_and)
            cc = main.tile([P, NJ], i32)
            nc.vector.tensor_tensor(out=cc, in0=c0, in1=zi, op=A.bitwise_or)
            cf = main.tile([P, NJ], f32)
            nc.vector.tensor_copy(cf, cc)

            # ---- z1 build + store per 2-slot chunk, Vector/GpSimd alternating ----
            for ch in range(4):
                js = ch * 2
                eng = nc.vector
                zt = main.tile([P, 2, V], f32, tag=f"zt{ch}")
                eng.tensor_tensor(
                    out=zt,
                    in0=bcast_mid(iota_f, 2),
                    in1=bcast_last(cf[:, js:js + 2], V),
                    op=A.is_equal,
                )
                nc.scalar.dma_start(out=out_r[:, js:js + 2, V:2 * V], in_=zt)

    nc.finalize()
    return nc


# Test-harness hooks: extra kwargs for run_bass_kernel_spmd (e.g. trace=True)
# and the last BassKernelResults for profiling. Unused when graded.
RUN_KWARGS: dict = {}
LAST_RESULTS = None


def kernel(**inputs) -> np.ndarray:
    global LAST_RESULTS
    from concourse.bass_utils import run_bass_kernel_spmd

    x = np.ascontiguousarray(np.asarray(inputs["inputs"], dtype=np.float32))
    W1 = np.asarray(inputs["W1"], dtype=np.float32)
    b1 = np.asarray(inputs["b1"], dtype=np.float32)
    W2 = np.asarray(inputs["W2"], dtype=np.float32)
    b2 = np.asarray(inputs["b2"], dtype=np.float32)

    # pure layout marshalling into one per-partition weight buffer
    wb = np.zeros((P, WB_COLS), np.float32)
    wb[:, W1T_OFF:W1T_OFF + KH * V] = (
        W1.T.reshape(KH, P, V).transpose(1, 0, 2).reshape(P, KH * V)
    )
    wb[:, B1_OFF:B1_OFF + KH] = b1.reshape(KH, P).T
    wb[:, W2_OFF:W2_OFF + KH * 2 * V] = (
        W2.reshape(KH, P, 2 * V).transpose(1, 0, 2).reshape(P, KH * 2 * V)
    )
    wb[:, B2_OFF:B2_OFF + 2 * V] = b2.reshape(1, 2 * V)

    B = x.shape[0]
    rows = B // N_CORES
    nc = build_bass(rows)

    shards = np.split(x, N_CORES, axis=0)
    in_maps = [{"x": s, "wb": wb} for s in shards]
    res = run_bass_kernel_spmd(nc, in_maps, list(range(N_CORES)), **RUN_KWARGS)
    LAST_RESULTS = res
    return np.concatenate([r["out"] for r in res.results], axis=0)


# revision 12
# speedup vs baseline: 1.0723x; 1.0723x over previous
"""DiscreteBipartiteFlow forward on 8 trn2 NeuronCores.

Math: inputs rows are exact one-hots (x0|x1). net = relu(x0@W1+b1)@W2+b2
only depends on i0=argmax(x0), so precompute on device the [V, 2V] table
NET = relu(W1+b1)@W2+b2 and its per-row argmaxes L[i], S[i]. For one-hot
x1 with index a1, z1 = one_hot((L[i0] + a1*S[i0]) mod V) (zero row when
S[i0]==0, scale index 0 being excluded). Output = [x0 | z1].

Per-core structure (1024 rows, 8 rows per partition):
 - SP ring: weights (w1t+b1+b2row first, then w2 per-k chunks so the
   fp32 NET matmul streams). ACT ring: x load + half the stores.
 - b2 folded into PSUM via tiny ones-row matmuls (start=True) before
   the NET accumulation; argmax (max8/max_index) reads PSUM directly.
 - S-head matmuls first: scale argmax + derived scalars hide under the
   L-head matmuls. PE warmup matmuls raise the tensor-engine pstate
   before the real table matmul.
 - pack table pk[i] = S + 128*L + 16384*[S==0] materialized along the
   free axis as PSUM via TWO accumulating bf16 matmuls (lhsT = value
   replicated per partition, rhs = identity): 128*L+16384*ZF and S are
   both exactly representable in bf16. comb[i0] per row then comes from
   8 scalar_tensor_tensor dot-products (x0 one-hot . pk), a1 likewise
   via dots against iota while the PE is busy.
 - int unpack (dual-op shifts/masks), z1 = is_equal(iota, c) written
   straight into a [P,8,2V] out tile whose x0 half was copied by the
   ACT engine; four [P,2,2V] stores alternate SP/ACT rings.
Data-parallel over 8 cores; weights replicated (host marshalling only).
"""

import numpy as np

V = 128
H = 512
N_CORES = 8
P = 128
NJ = 8               # row slots per partition
KH = H // P          # 4 contraction chunks

W1T_OFF = 0          # [P, 512]  w1t[p, k*128+i] = W1[i, 128k+p]
B1_OFF = 512         # [P, 4]    b1[p, k] = b1[128k+p]
B2_OFF = 516         # [P, 256]  b2 (used on partition 0 only)
W2_OFF = 772         # [P, 1024] w2[p, k*256+c] = W2[128k+p, c]
WB_COLS = 1796


def build_bass(rows: int):
    """Build the single-core Bass program for a [rows, 2V] batch shard."""
    import concourse.bacc as bacc
    import concourse.bass as bass
    import concourse.tile as tile
    from concourse import mybir

    f32 = mybir.dt.float32
    bf16 = mybir.dt.bfloat16
    i32 = mybir.dt.int32
    u32 = mybir.dt.uint32
    A = mybir.AluOpType
    AF = mybir.ActivationFunctionType

    assert rows == P * NJ

    nc = bacc.Bacc(None)
    x = nc.declare_dram_parameter("x", [rows, 2 * V], f32, isOutput=False)
    wb = nc.declare_dram_parameter("wb", [P, WB_COLS], f32, isOutput=False)
    out = nc.declare_dram_parameter("out", [rows, 2 * V], f32, isOutput=True)

    x_r = x.rearrange("(p j) n -> p j n", j=NJ)
    out_r = out.rearrange("(p j) n -> p j n", j=NJ)

    def bcast_mid(t_ap, reps):
        return bass.AP(
            tensor=t_ap.tensor, offset=t_ap.offset,
            ap=[t_ap.ap[0], [0, reps]] + list(t_ap.ap[1:]),
        )

    def bcast_last(t_ap, reps):
        return bass.AP(
            tensor=t_ap.tensor, offset=t_ap.offset,
            ap=list(t_ap.ap) + [[0, reps]],
        )

    with tile.TileContext(nc) as tc:
        with (
            tc.tile_pool(name="main", bufs=1) as main,
            tc.tile_pool(name="psum_net", bufs=1, space="PSUM") as psum_net,
            tc.tile_pool(name="psum_pb", bufs=1, space="PSUM") as psum_pb,
            tc.tile_pool(name="psum_wu", bufs=1, space="PSUM") as psum_wu,
        ):
            # ---- input DMAs: weights on SP ring, x on ACT ring ----
            wb_sb = main.tile([P, WB_COLS], f32)
            nc.sync.dma_start(out=wb_sb[:, 0:W2_OFF], in_=wb[:, 0:W2_OFF])
            for k in range(KH):
                o = W2_OFF + k * 2 * V
                nc.sync.dma_start(out=wb_sb[:, o:o + 2 * V], in_=wb[:, o:o + 2 * V])
            xw = main.tile([P, NJ, 2 * V], f32)
            nc.scalar.dma_start(out=xw, in_=x_r[:, :, :])

            w1t = wb_sb[:, W1T_OFF:W1T_OFF + KH * V].rearrange("p (k i) -> p k i", k=KH)
            b1s = wb_sb[:, B1_OFF:B1_OFF + KH]
            b2r = wb_sb[0:1, B2_OFF:B2_OFF + 2 * V]
            w2 = wb_sb[:, W2_OFF:W2_OFF + KH * 2 * V].rearrange("p (k c) -> p k c", k=KH)

            # ---- on-device constants ----
            iota_f = main.tile([P, V], f32)
            nc.gpsimd.iota(iota_f, pattern=[[1, V]], base=0, channel_multiplier=0,
                           allow_small_or_imprecise_dtypes=True)
            ipart_f = main.tile([P, 1], f32)
            nc.gpsimd.iota(ipart_f, pattern=[[0, 1]], base=0, channel_multiplier=1,
                           allow_small_or_imprecise_dtypes=True)
            ident_bf = main.tile([P, V], bf16)
            nc.vector.tensor_tensor(out=ident_bf, in0=iota_f,
                                    in1=bcast_last(ipart_f, V), op=A.is_equal)
            ones1 = main.tile([1, V], f32)
            nc.vector.memset(ones1, 1.0)

            # ---- PE warmup: raise pstate before the fp32 NET matmul ----
            warm_ps = psum_wu.tile([P, V], f32)
            for w in range(6):
                nc.tensor.matmul(warm_ps, lhsT=ident_bf, rhs=ident_bf,
                                 start=True, stop=True)

            # ---- table phase ----
            hT = main.tile([P, KH, V], f32)
            for k in range(KH):
                nc.scalar.activation(out=hT[:, k, :], in_=w1t[:, k, :], func=AF.Relu,
                                     bias=b1s[:, k:k + 1], scale=1.0)
            net_ps = psum_net.tile([P, 2 * V], f32)
            # S-head (cols V:2V) first; b2 folded in via ones-row matmul
            for head in (1, 0):
                nc.tensor.matmul(
                    net_ps[:, head * V:(head + 1) * V], lhsT=ones1,
                    rhs=b2r[:, head * V:(head + 1) * V], start=True, stop=False,
                )
                for k in range(KH):
                    nc.tensor.matmul(
                        net_ps[:, head * V:(head + 1) * V], lhsT=hT[:, k, :],
                        rhs=w2[:, k, head * V:(head + 1) * V],
                        start=False, stop=(k == KH - 1),
                    )

            # ---- argmax per head straight off PSUM (S first) ----
            idx = []
            for head in (1, 0):
                m8 = main.tile([P, 8], f32, tag=f"m8{head}")
                nc.vector.max(m8, net_ps[:, head * V:(head + 1) * V])
                ix = main.tile([P, 8], u32, tag=f"ix{head}")
                nc.vector.max_index(ix, m8, net_ps[:, head * V:(head + 1) * V])
                idx.append(ix)
            idxS, idxL = idx

            # scale-side scalars (hidden under L-head matmuls)
            sf = main.tile([P, 1], f32)
            nc.vector.tensor_copy(sf, idxS[:, 0:1])
            z16 = main.tile([P, 1], f32)
            nc.vector.tensor_scalar(out=z16, in0=sf, scalar1=0.5, scalar2=None,
                                    op0=A.is_lt)
            pkS_rep = main.tile([P, V], bf16)
            nc.vector.tensor_copy(pkS_rep, bcast_last(sf[:, 0:1], V))

            # L-side: pk_hi = 128*L + 16384*[S==0] (exact in bf16)
            lf = main.tile([P, 1], f32)
            nc.vector.tensor_copy(lf, idxL[:, 0:1])
            pkhi = main.tile([P, 1], f32)
            nc.vector.tensor_scalar(out=pkhi, in0=z16, scalar1=float(V * V),
                                    scalar2=lf[:, 0:1], op0=A.mult, op1=A.add)
            pkhi128 = main.tile([P, V], bf16)
            nc.vector.tensor_scalar(out=pkhi128, in0=bcast_last(pkhi[:, 0:1], V),
                                    scalar1=float(V), scalar2=None, op0=A.mult)

            # packB[m, v] = pk[v] via two accumulating bf16 matmuls
            packB = psum_pb.tile([P, V], f32)
            nc.tensor.matmul(packB, lhsT=pkS_rep, rhs=ident_bf, start=True, stop=False)
            nc.tensor.matmul(packB, lhsT=pkhi128, rhs=ident_bf, start=False, stop=True)

            # ---- a1 dots on Vector while the PE runs NET (program order
            # after argmax ops so the scheduler prefers argmax when ready) --
            a1f = main.tile([P, NJ], f32)
            dot_scr = main.tile([P, NJ, V], f32)
            for j in range(NJ):
                nc.vector.scalar_tensor_tensor(
                    out=dot_scr[:, j, :], in0=xw[:, j, V:2 * V], scalar=1.0,
                    in1=iota_f, op0=A.bypass, op1=A.mult,
                    accum_out=a1f[:, j:j + 1],
                )
            a1i = main.tile([P, NJ], i32)
            nc.gpsimd.tensor_copy(a1i, a1f)

            # ---- x0 passthrough into the output tile (ACT, hidden) ----
            out_sb = main.tile([P, NJ, 2 * V], f32)
            for h in range(2):
                nc.scalar.copy(out_sb[:, 4 * h:4 * h + 4, 0:V],
                               xw[:, 4 * h:4 * h + 4, 0:V])

            # ---- comb dots: comb[p,j] = pk[i0] ----
            comb_scr = main.tile([P, NJ, V], f32)
            combf = main.tile([P, NJ], f32)
            for j in range(NJ):
                nc.vector.scalar_tensor_tensor(
                    out=comb_scr[:, j, :], in0=xw[:, j, 0:V], scalar=1.0,
                    in1=packB, op0=A.bypass, op1=A.mult,
                    accum_out=combf[:, j:j + 1],
                )

            # ---- unpack: c = ((L + a1*S) & 127) | 128*[S==0] ----
            ci = main.tile([P, NJ], i32)
            nc.vector.tensor_copy(ci, combf)
            si = main.tile([P, NJ], i32)
            nc.vector.tensor_scalar(out=si, in0=ci, scalar1=V - 1, scalar2=None,
                                    op0=A.bitwise_and)
            sa = main.tile([P, NJ], i32)
            nc.gpsimd.tensor_mul(sa, si, a1i)
            li = main.tile([P, NJ], i32)
            nc.vector.tensor_scalar(out=li, in0=ci, scalar1=7, scalar2=V - 1,
                                    op0=A.arith_shift_right, op1=A.bitwise_and)
            zi = main.tile([P, NJ], i32)
            nc.vector.tensor_scalar(out=zi, in0=ci, scalar1=7, scalar2=V,
                                    op0=A.arith_shift_right, op1=A.bitwise_and)
            t2 = main.tile([P, NJ], i32)
            nc.vector.tensor_add(t2, sa, li)
            c0 = main.tile([P, NJ], i32)
            nc.vector.tensor_scalar(out=c0, in0=t2, scalar1=V - 1, scalar2=None,
                                    op0=A.bitwise_and)
            cc = main.tile([P, NJ], i32)
            nc.vector.tensor_tensor(out=cc, in0=c0, in1=zi, op=A.bitwise_or)
            cf = main.tile([P, NJ], f32)
            nc.vector.tensor_copy(cf, cc)

            # ---- z1 into out_sb + store per 2-slot chunk (SP/ACT rings) ----
            for ch in range(4):
                js = ch * 2
                nc.vector.tensor_tensor(
                    out=out_sb[:, js:js + 2, V:2 * V],
                    in0=bcast_mid(iota_f, 2),
                    in1=bcast_last(cf[:, js:js + 2], V),
                    op=A.is_equal,
                )
                eng = nc.sync if ch % 2 == 0 else nc.scalar
                eng.dma_start(out=out_r[:, js:js + 2, :], in_=out_sb[:, js:js + 2, :])

    nc.finalize()
    return nc


# Test-harness hooks: extra kwargs for run_bass_kernel_spmd (e.g. trace=True)
# and the last BassKernelResults for profiling. Unused when graded.
RUN_KWARGS: dict = {}
LAST_RESULTS = None


def kernel(**inputs) -> np.ndarray:
    global LAST_RESULTS
    from concourse.bass_utils import run_bass_kernel_spmd

    x = np.ascontiguousarray(np.asarray(inputs["inputs"], dtype=np.float32))
    W1 = np.asarray(inputs["W1"], dtype=np.float32)
    b1 = np.asarray(inputs["b1"], dtype=np.float32)
    W2 = np.asarray(inputs["W2"], dtype=np.float32)
    b2 = np.asarray(inputs["b2"], dtype=np.float32)

    # pure layout marshalling into one per-partition weight buffer
    wb = np.zeros((P, WB_COLS), np.float32)
    wb[:, W1T_OFF:W1T_OFF + KH * V] = (
        W1.T.reshape(KH, P, V).transpose(1, 0, 2).reshape(P, KH * V)
    )
    wb[:, B1_OFF:B1_OFF + KH] = b1.reshape(KH, P).T
    wb[:, B2_OFF:B2_OFF + 2 * V] = b2.reshape(1, 2 * V)
    wb[:, W2_OFF:W2_OFF + KH * 2 * V] = (
        W2.reshape(KH, P, 2 * V).transpose(1, 0, 2).reshape(P, KH * 2 * V)
    )

    B = x.shape[0]
    rows = B // N_CORES
    nc = build_bass(rows)

    shards = np.split(x, N_CORES, axis=0)
    in_maps = [{"x": s, "wb": wb} for s in shards]
    res = run_bass_kernel_spmd(nc, in_maps, list(range(N_CORES)), **RUN_KWARGS)
    LAST_RESULTS = res
    return np.concatenate([r["out"] for r in res.results], axis=0)


# revision 14
# speedup vs baseline: 1.1375x; 1.0608x over previous
"""DiscreteBipartiteFlow forward on 8 trn2 NeuronCores.

Math: inputs rows are exact one-hots (x0|x1). net = relu(x0@W1+b1)@W2+b2
only depends on i0=argmax(x0), so precompute on device the [V, 2V] table
NET = relu(W1+b1)@W2+b2 and its per-row argmaxes L[i], S[i]. For one-hot
x1 with index a1, z1 = one_hot((L[i0] + a1*S[i0]) mod V) (zero row when
S[i0]==0, scale index 0 being excluded). Output = [x0 | z1].

Per-core structure (1024 rows, 8 rows per partition):
 - ALL loads on the SP ring in priority order (w1t+b1+b2row, w2 per-k,
   then x) — the 16 DMA engines are shared between rings, so ring order
   is the only way to guarantee the NET matmul's weights land first.
   Stores alternate SP/ACT rings.
 - b2 folded into PSUM via ones-row matmuls; argmax reads PSUM.
   S-head matmuls before L-head so scale-side scalars hide under them.
   PE warmup matmuls raise the tensor-engine pstate first.
 - ONE combined lookup table in PSUM [P, 2V]: cols 0:V hold
   pk[v] = 128*S + 16384*L + 2^21*[S==0] (built by TWO accumulating
   bf16 matmuls — both addends are exactly representable in bf16 —
   with lhsT = per-partition-replicated values, rhs = identity) and
   cols V:2V hold iota (ones-row x iota-row matmul). Then ONE 256-wide
   dot per row-slot (scalar_tensor_tensor accum) over the FULL input
   row yields comb = pk[i0] + a1 exactly in fp32.
 - int unpack (dual-op shifts/masks, multiply on GpSimd), z1 =
   is_equal(iota, c) written into a [P,8,2V] out tile whose x0 half the
   ACT engine copied; four [P,2,2V] stores alternate SP/ACT rings.
Data-parallel over 8 cores; weights replicated (host marshalling only).
"""

import numpy as np

V = 128
H = 512
N_CORES = 8
P = 128
NJ = 8               # row slots per partition
KH = H // P          # 4 contraction chunks

W1T_OFF = 0          # [P, 512]  w1t[p, k*128+i] = W1[i, 128k+p]
B1_OFF = 512         # [P, 4]    b1[p, k] = b1[128k+p]
B2_OFF = 516         # [P, 256]  b2 (used on partition 0 only)
W2_OFF = 772         # [P, 1024] w2[p, k*256+c] = W2[128k+p, c]
WB_COLS = 1796


def build_bass(rows: int):
    """Build the single-core Bass program for a [rows, 2V] batch shard."""
    import concourse.bacc as bacc
    import concourse.bass as bass
    import concourse.tile as tile
    from concourse import mybir

    f32 = mybir.dt.float32
    bf16 = mybir.dt.bfloat16
    i32 = mybir.dt.int32
    u32 = mybir.dt.uint32
    A = mybir.AluOpType
    AF = mybir.ActivationFunctionType

    assert rows == P * NJ

    nc = bacc.Bacc(None)
    x = nc.declare_dram_parameter("x", [rows, 2 * V], f32, isOutput=False)
    wb = nc.declare_dram_parameter("wb", [P, WB_COLS], f32, isOutput=False)
    out = nc.declare_dram_parameter("out", [rows, 2 * V], f32, isOutput=True)

    x_r = x.rearrange("(p j) n -> p j n", j=NJ)
    out_r = out.rearrange("(p j) n -> p j n", j=NJ)

    def bcast_mid(t_ap, reps):
        return bass.AP(
            tensor=t_ap.tensor, offset=t_ap.offset,
            ap=[t_ap.ap[0], [0, reps]] + list(t_ap.ap[1:]),
        )

    def bcast_last(t_ap, reps):
        return bass.AP(
            tensor=t_ap.tensor, offset=t_ap.offset,
            ap=list(t_ap.ap) + [[0, reps]],
        )

    with tile.TileContext(nc) as tc:
        with (
            tc.tile_pool(name="main", bufs=1) as main,
            tc.tile_pool(name="psum_net", bufs=1, space="PSUM") as psum_net,
            tc.tile_pool(name="psum_pb", bufs=1, space="PSUM") as psum_pb,
            tc.tile_pool(name="psum_wu", bufs=1, space="PSUM") as psum_wu,
        ):
            # ---- loads, all SP ring, priority order ----
            wb_sb = main.tile([P, WB_COLS], f32)
            nc.sync.dma_start(out=wb_sb[:, 0:W2_OFF], in_=wb[:, 0:W2_OFF])
            for k in range(KH):
                o = W2_OFF + k * 2 * V
                nc.sync.dma_start(out=wb_sb[:, o:o + 2 * V], in_=wb[:, o:o + 2 * V])
            xw = main.tile([P, NJ, 2 * V], f32)
            nc.sync.dma_start(out=xw[:, 0:4, :], in_=x_r[:, 0:4, :])
            nc.sync.dma_start(out=xw[:, 4:8, :], in_=x_r[:, 4:8, :])

            w1t = wb_sb[:, W1T_OFF:W1T_OFF + KH * V].rearrange("p (k i) -> p k i", k=KH)
            b1s = wb_sb[:, B1_OFF:B1_OFF + KH]
            b2r = wb_sb[0:1, B2_OFF:B2_OFF + 2 * V]
            w2 = wb_sb[:, W2_OFF:W2_OFF + KH * 2 * V].rearrange("p (k c) -> p k c", k=KH)

            # ---- on-device constants ----
            iota_f = main.tile([P, V], f32)
            nc.gpsimd.iota(iota_f, pattern=[[1, V]], base=0, channel_multiplier=0,
                           allow_small_or_imprecise_dtypes=True)
            ipart_f = main.tile([P, 1], f32)
            nc.gpsimd.iota(ipart_f, pattern=[[0, 1]], base=0, channel_multiplier=1,
                           allow_small_or_imprecise_dtypes=True)
            ident_bf = main.tile([P, V], bf16)
            nc.vector.tensor_tensor(out=ident_bf, in0=iota_f,
                                    in1=bcast_last(ipart_f, V), op=A.is_equal)
            ones1 = main.tile([1, V], f32)
            nc.vector.memset(ones1, 1.0)

            # ---- PE warmup: raise pstate before the fp32 NET matmul ----
            warm_ps = psum_wu.tile([P, V], f32)
            for w in range(6):
                nc.tensor.matmul(warm_ps, lhsT=ident_bf, rhs=ident_bf,
                                 start=True, stop=True)

            # combined lookup table, iota half (constants only, runs early)
            packB = psum_pb.tile([P, 2 * V], f32)
            nc.tensor.matmul(packB[:, V:2 * V], lhsT=ones1, rhs=iota_f[0:1, :],
                             start=True, stop=True)

            # ---- table phase ----
            hT = main.tile([P, KH, V], f32)
            for k in range(KH):
                nc.scalar.activation(out=hT[:, k, :], in_=w1t[:, k, :], func=AF.Relu,
                                     bias=b1s[:, k:k + 1], scale=1.0)
            net_ps = psum_net.tile([P, 2 * V], f32)
            # S-head (cols V:2V) first; b2 folded in via ones-row matmul
            for head in (1, 0):
                nc.tensor.matmul(
                    net_ps[:, head * V:(head + 1) * V], lhsT=ones1,
                    rhs=b2r[:, head * V:(head + 1) * V], start=True, stop=False,
                )
                for k in range(KH):
                    nc.tensor.matmul(
                        net_ps[:, head * V:(head + 1) * V], lhsT=hT[:, k, :],
                        rhs=w2[:, k, head * V:(head + 1) * V],
                        start=False, stop=(k == KH - 1),
                    )

            # ---- argmax per head straight off PSUM (S first) ----
            idx = []
            for head in (1, 0):
                m8 = main.tile([P, 8], f32, tag=f"m8{head}")
                nc.vector.max(m8, net_ps[:, head * V:(head + 1) * V])
                ix = main.tile([P, 8], u32, tag=f"ix{head}")
                nc.vector.max_index(ix, m8, net_ps[:, head * V:(head + 1) * V])
                idx.append(ix)
            idxS, idxL = idx

            # scale-side scalars (hidden under L-head matmuls):
            # pkS128 = 128*S; z21 = [S==0]
            sf = main.tile([P, 1], f32)
            nc.vector.tensor_copy(sf, idxS[:, 0:1])
            z21 = main.tile([P, 1], f32)
            nc.vector.tensor_scalar(out=z21, in0=sf, scalar1=0.5, scalar2=None,
                                    op0=A.is_lt)
            pkS128 = main.tile([P, V], bf16)
            nc.vector.tensor_scalar(out=pkS128, in0=bcast_last(sf[:, 0:1], V),
                                    scalar1=float(V), scalar2=None, op0=A.mult)

            # L-side: pk_hi = 16384*L + 2^21*[S==0] (exact in bf16)
            lf = main.tile([P, 1], f32)
            nc.vector.tensor_copy(lf, idxL[:, 0:1])
            pkhi = main.tile([P, 1], f32)
            nc.vector.tensor_scalar(out=pkhi, in0=z21, scalar1=float(V), scalar2=lf[:, 0:1],
                                    op0=A.mult, op1=A.add)
            pkhi_rep = main.tile([P, V], bf16)
            nc.vector.tensor_scalar(out=pkhi_rep, in0=bcast_last(pkhi[:, 0:1], V),
                                    scalar1=float(V * V), scalar2=None, op0=A.mult)

            # pk half of the lookup table via two accumulating bf16 matmuls
            nc.tensor.matmul(packB[:, 0:V], lhsT=pkS128, rhs=ident_bf,
                             start=True, stop=False)
            nc.tensor.matmul(packB[:, 0:V], lhsT=pkhi_rep, rhs=ident_bf,
                             start=False, stop=True)

            # ---- x0 passthrough into the output tile (ACT, hidden) ----
            out_sb = main.tile([P, NJ, 2 * V], f32)
            for h in range(2):
                nc.scalar.copy(out_sb[:, 4 * h:4 * h + 4, 0:V],
                               xw[:, 4 * h:4 * h + 4, 0:V])

            # ---- 256-wide dots: comb[p,j] = pk[i0] + a1, exact in fp32 ----
            dot_scr = main.tile([P, NJ, 2 * V], f32)
            combf = main.tile([P, NJ], f32)
            for j in range(NJ):
                nc.vector.scalar_tensor_tensor(
                    out=dot_scr[:, j, :], in0=xw[:, j, :], scalar=1.0,
                    in1=packB, op0=A.bypass, op1=A.mult,
                    accum_out=combf[:, j:j + 1],
                )

            # ---- unpack: c = ((L + a1*S) & 127) | 128*[S==0] ----
            ci = main.tile([P, NJ], i32)
            nc.vector.tensor_copy(ci, combf)
            a1x = main.tile([P, NJ], i32)
            nc.vector.tensor_scalar(out=a1x, in0=ci, scalar1=V - 1, scalar2=None,
                                    op0=A.bitwise_and)
            sx = main.tile([P, NJ], i32)
            nc.vector.tensor_scalar(out=sx, in0=ci, scalar1=7, scalar2=V - 1,
                                    op0=A.arith_shift_right, op1=A.bitwise_and)
            sa = main.tile([P, NJ], i32)
            nc.gpsimd.tensor_mul(sa, a1x, sx)
            li = main.tile([P, NJ], i32)
            nc.vector.tensor_scalar(out=li, in0=ci, scalar1=14, scalar2=V - 1,
                                    op0=A.arith_shift_right, op1=A.bitwise_and)
            zi = main.tile([P, NJ], i32)
            nc.vector.tensor_scalar(out=zi, in0=ci, scalar1=14, scalar2=V,
                                    op0=A.arith_shift_right, op1=A.bitwise_and)
            t2 = main.tile([P, NJ], i32)
            nc.vector.tensor_add(t2, sa, li)
            c0 = main.tile([P, NJ], i32)
            nc.vector.tensor_scalar(out=c0, in0=t2, scalar1=V - 1, scalar2=None,
                                    op0=A.bitwise_and)
            cc = main.tile([P, NJ], i32)
            nc.vector.tensor_tensor(out=cc, in0=c0, in1=zi, op=A.bitwise_or)
            cf = main.tile([P, NJ], f32)
            nc.vector.tensor_copy(cf, cc)

            # ---- z1 into out_sb + store per 2-slot chunk (SP/ACT rings) ----
            for ch in range(4):
                js = ch * 2
                nc.vector.tensor_tensor(
                    out=out_sb[:, js:js + 2, V:2 * V],
                    in0=bcast_mid(iota_f, 2),
                    in1=bcast_last(cf[:, js:js + 2], V),
                    op=A.is_equal,
                )
                eng = nc.sync if ch % 2 == 0 else nc.scalar
                eng.dma_start(out=out_r[:, js:js + 2, :], in_=out_sb[:, js:js + 2, :])

    nc.finalize()
    return nc


# Test-harness hooks: extra kwargs for run_bass_kernel_spmd (e.g. trace=True)
# and the last BassKernelResults for profiling. Unused when graded.
RUN_KWARGS: dict = {}
LAST_RESULTS = None


def kernel(**inputs) -> np.ndarray:
    global LAST_RESULTS
    from concourse.bass_utils import run_bass_kernel_spmd

    x = np.ascontiguousarray(np.asarray(inputs["inputs"], dtype=np.float32))
    W1 = np.asarray(inputs["W1"], dtype=np.float32)
    b1 = np.asarray(inputs["b1"], dtype=np.float32)
    W2 = np.asarray(inputs["W2"], dtype=np.float32)
    b2 = np.asarray(inputs["b2"], dtype=np.float32)

    # pure layout marshalling into one per-partition weight buffer
    wb = np.zeros((P, WB_COLS), np.float32)
    wb[:, W1T_OFF:W1T_OFF + KH * V] = (
        W1.T.reshape(KH, P, V).transpose(1, 0, 2).reshape(P, KH * V)
    )
    wb[:, B1_OFF:B1_OFF + KH] = b1.reshape(KH, P).T
    wb[:, B2_OFF:B2_OFF + 2 * V] = b2.reshape(1, 2 * V)
    wb[:, W2_OFF:W2_OFF + KH * 2 * V] = (
        W2.reshape(KH, P, 2 * V).transpose(1, 0, 2).reshape(P, KH * 2 * V)
    )

    B = x.shape[0]
    rows = B // N_CORES
    nc = build_bass(rows)

    shards = np.split(x, N_CORES, axis=0)
    in_maps = [{"x": s, "wb": wb} for s in shards]
    res = run_bass_kernel_spmd(nc, in_maps, list(range(N_CORES)), **RUN_KWARGS)
    LAST_RESULTS = res
    return np.concatenate([r["out"] for r in res.results], axis=0)


# revision 15
# speedup vs baseline: 1.1982x; 1.0533x over previous
"""DiscreteBipartiteFlow forward on 8 trn2 NeuronCores.

Math: inputs rows are exact one-hots (x0|x1). net = relu(x0@W1+b1)@W2+b2
only depends on i0=argmax(x0), so precompute on device the [V, 2V] table
NET = relu(W1+b1)@W2+b2 and its per-row argmaxes L[i], S[i]. For one-hot
x1 with index a1, z1 = one_hot((L[i0] + a1*S[i0]) mod V) (zero row when
S[i0]==0, scale index 0 being excluded). Output = [x0 | z1].

Per-core structure (1024 rows, 8 rows per partition):
 - ALL loads on the SP ring in priority order (w1t chunk0 + biases
   first so relu/NET start ASAP, then w2 per-k, then x) — the 16 DMA
   engines are shared between rings, so ring order decides arrival.
   Stores alternate SP/ACT rings.
 - b2 folded into PSUM via a ones-row matmul heading the NET
   accumulation; full-width (N=2V) fp32 chunk matmuls; argmax
   (max8/max_index) reads PSUM directly.
 - ONE combined lookup table in PSUM [P, 2V]: cols 0:V hold
   pk[v] = 128*S + 16384*L + 2^21*[S==0] (TWO accumulating bf16
   matmuls — both addends exactly representable in bf16 — with
   lhsT = per-partition-replicated values, rhs = identity), cols V:2V
   hold iota (ones-row x iota-row matmul, runs at kernel start). ONE
   256-wide dot per row-slot (scalar_tensor_tensor accum) over the
   full input row then yields comb = pk[i0] + a1 exactly in fp32.
 - int unpack (dual-op shifts/masks, multiply on GpSimd), z1 =
   is_equal(iota, c) written into a [P,8,2V] out tile whose x0 half the
   ACT engine copied; four [P,2,2V] stores alternate SP/ACT rings.
Data-parallel over 8 cores; weights replicated (host marshalling only).
"""

import numpy as np

V = 128
H = 512
N_CORES = 8
P = 128
NJ = 8               # row slots per partition
KH = H // P          # 4 contraction chunks

# host-marshalled weight buffer layout (fp32 columns per partition)
W1K0_OFF = 0         # [P, 128]  w1t k0
B1_OFF = 128         # [P, 4]    b1[p, k] = b1[128k+p]
B2_OFF = 132         # [P, 256]  b2 (used on partition 0 only)
W1R_OFF = 388        # [P, 384]  w1t k1..k3
W2_OFF = 772         # [P, 1024] w2[p, k*256+c] = W2[128k+p, c]
WB_COLS = 1796


def build_bass(rows: int):
    """Build the single-core Bass program for a [rows, 2V] batch shard."""
    import concourse.bacc as bacc
    import concourse.bass as bass
    import concourse.tile as tile
    from concourse import mybir

    f32 = mybir.dt.float32
    bf16 = mybir.dt.bfloat16
    i32 = mybir.dt.int32
    u32 = mybir.dt.uint32
    A = mybir.AluOpType
    AF = mybir.ActivationFunctionType

    assert rows == P * NJ

    nc = bacc.Bacc(None)
    x = nc.declare_dram_parameter("x", [rows, 2 * V], f32, isOutput=False)
    wb = nc.declare_dram_parameter("wb", [P, WB_COLS], f32, isOutput=False)
    out = nc.declare_dram_parameter("out", [rows, 2 * V], f32, isOutput=True)

    x_r = x.rearrange("(p j) n -> p j n", j=NJ)
    out_r = out.rearrange("(p j) n -> p j n", j=NJ)

    def bcast_mid(t_ap, reps):
        return bass.AP(
            tensor=t_ap.tensor, offset=t_ap.offset,
            ap=[t_ap.ap[0], [0, reps]] + list(t_ap.ap[1:]),
        )

    def bcast_last(t_ap, reps):
        return bass.AP(
            tensor=t_ap.tensor, offset=t_ap.offset,
            ap=list(t_ap.ap) + [[0, reps]],
        )

    with tile.TileContext(nc) as tc:
        with (
            tc.tile_pool(name="main", bufs=1) as main,
            tc.tile_pool(name="psum_net", bufs=1, space="PSUM") as psum_net,
            tc.tile_pool(name="psum_pb", bufs=1, space="PSUM") as psum_pb,
        ):
            # ---- loads, all SP ring, priority order ----
            wb_sb = main.tile([P, WB_COLS], f32)
            nc.sync.dma_start(out=wb_sb[:, 0:W1R_OFF], in_=wb[:, 0:W1R_OFF])
            nc.sync.dma_start(out=wb_sb[:, W1R_OFF:W2_OFF], in_=wb[:, W1R_OFF:W2_OFF])
            for k in range(KH):
                o = W2_OFF + k * 2 * V
                nc.sync.dma_start(out=wb_sb[:, o:o + 2 * V], in_=wb[:, o:o + 2 * V])
            xw = main.tile([P, NJ, 2 * V], f32)
            nc.sync.dma_start(out=xw[:, 0:4, :], in_=x_r[:, 0:4, :])
            nc.sync.dma_start(out=xw[:, 4:8, :], in_=x_r[:, 4:8, :])

            b1s = wb_sb[:, B1_OFF:B1_OFF + KH]
            b2r = wb_sb[0:1, B2_OFF:B2_OFF + 2 * V]
            w2 = wb_sb[:, W2_OFF:W2_OFF + KH * 2 * V].rearrange("p (k c) -> p k c", k=KH)

            def w1tk(k):
                if k == 0:
                    return wb_sb[:, W1K0_OFF:W1K0_OFF + V]
                o = W1R_OFF + (k - 1) * V
                return wb_sb[:, o:o + V]

            # ---- on-device constants ----
            iota_f = main.tile([P, V], f32)
            nc.gpsimd.iota(iota_f, pattern=[[1, V]], base=0, channel_multiplier=0,
                           allow_small_or_imprecise_dtypes=True)
            ipart_f = main.tile([P, 1], f32)
            nc.gpsimd.iota(ipart_f, pattern=[[0, 1]], base=0, channel_multiplier=1,
                           allow_small_or_imprecise_dtypes=True)
            ident_bf = main.tile([P, V], bf16)
            nc.vector.tensor_tensor(out=ident_bf, in0=iota_f,
                                    in1=bcast_last(ipart_f, V), op=A.is_equal)
            ones1 = main.tile([1, V], f32)
            nc.vector.memset(ones1, 1.0)

            # combined lookup table, iota half (constants only, runs early)
            packB = psum_pb.tile([P, 2 * V], f32)
            nc.tensor.matmul(packB[:, V:2 * V], lhsT=ones1, rhs=iota_f[0:1, :],
                             start=True, stop=True)

            # ---- table phase: full-width fp32 chunks, b2 heads the group --
            hT = main.tile([P, KH, V], f32)
            for k in range(KH):
                nc.scalar.activation(out=hT[:, k, :], in_=w1tk(k), func=AF.Relu,
                                     bias=b1s[:, k:k + 1], scale=1.0)
            net_ps = psum_net.tile([P, 2 * V], f32)
            nc.tensor.matmul(net_ps, lhsT=ones1, rhs=b2r, start=True, stop=False)
            for k in range(KH):
                nc.tensor.matmul(net_ps, lhsT=hT[:, k, :], rhs=w2[:, k, :],
                                 start=False, stop=(k == KH - 1))

            # ---- argmax per head straight off PSUM (S head first) ----
            idx = []
            for head in (1, 0):
                m8 = main.tile([P, 8], f32, tag=f"m8{head}")
                nc.vector.max(m8, net_ps[:, head * V:(head + 1) * V])
                ix = main.tile([P, 8], u32, tag=f"ix{head}")
                nc.vector.max_index(ix, m8, net_ps[:, head * V:(head + 1) * V])
                idx.append(ix)
                if head == 1:
                    # scale-side scalars: pkS128 = 128*S; z21 = [S==0]
                    sf = main.tile([P, 1], f32)
                    nc.vector.tensor_copy(sf, ix[:, 0:1])
                    z21 = main.tile([P, 1], f32)
                    nc.vector.tensor_scalar(out=z21, in0=sf, scalar1=0.5,
                                            scalar2=None, op0=A.is_lt)
                    pkS128 = main.tile([P, V], bf16)
                    nc.vector.tensor_scalar(out=pkS128,
                                            in0=bcast_last(sf[:, 0:1], V),
                                            scalar1=float(V), scalar2=None,
                                            op0=A.mult)
            idxS, idxL = idx

            # L-side: pk_hi = 16384*L + 2^21*[S==0] (exact in bf16)
            lf = main.tile([P, 1], f32)
            nc.vector.tensor_copy(lf, idxL[:, 0:1])
            pkhi = main.tile([P, 1], f32)
            nc.vector.tensor_scalar(out=pkhi, in0=z21, scalar1=float(V),
                                    scalar2=lf[:, 0:1], op0=A.mult, op1=A.add)
            pkhi_rep = main.tile([P, V], bf16)
            nc.vector.tensor_scalar(out=pkhi_rep, in0=bcast_last(pkhi[:, 0:1], V),
                                    scalar1=float(V * V), scalar2=None, op0=A.mult)

            # pk half of the lookup table via two accumulating bf16 matmuls
            nc.tensor.matmul(packB[:, 0:V], lhsT=pkS128, rhs=ident_bf,
                             start=True, stop=False)
            nc.tensor.matmul(packB[:, 0:V], lhsT=pkhi_rep, rhs=ident_bf,
                             start=False, stop=True)

            # ---- x0 passthrough into the output tile (ACT, hidden) ----
            out_sb = main.tile([P, NJ, 2 * V], f32)
            for h in range(2):
                nc.scalar.copy(out_sb[:, 4 * h:4 * h + 4, 0:V],
                               xw[:, 4 * h:4 * h + 4, 0:V])

            # ---- 256-wide dots: comb[p,j] = pk[i0] + a1, exact in fp32 ----
            dot_scr = main.tile([P, NJ, 2 * V], f32)
            combf = main.tile([P, NJ], f32)
            for j in range(NJ):
                nc.vector.scalar_tensor_tensor(
                    out=dot_scr[:, j, :], in0=xw[:, j, :], scalar=1.0,
                    in1=packB, op0=A.bypass, op1=A.mult,
                    accum_out=combf[:, j:j + 1],
                )

            # ---- unpack: c = ((L + a1*S) & 127) | 128*[S==0] ----
            ci = main.tile([P, NJ], i32)
            nc.vector.tensor_copy(ci, combf)
            a1x = main.tile([P, NJ], i32)
            nc.vector.tensor_scalar(out=a1x, in0=ci, scalar1=V - 1, scalar2=None,
                                    op0=A.bitwise_and)
            sx = main.tile([P, NJ], i32)
            nc.vector.tensor_scalar(out=sx, in0=ci, scalar1=7, scalar2=V - 1,
                                    op0=A.arith_shift_right, op1=A.bitwise_and)
            sa = main.tile([P, NJ], i32)
            nc.gpsimd.tensor_mul(sa, a1x, sx)
            li = main.tile([P, NJ], i32)
            nc.vector.tensor_scalar(out=li, in0=ci, scalar1=14, scalar2=V - 1,
                                    op0=A.arith_shift_right, op1=A.bitwise_and)
            zi = main.tile([P, NJ], i32)
            nc.vector.tensor_scalar(out=zi, in0=ci, scalar1=14, scalar2=V,
                                    op0=A.arith_shift_right, op1=A.bitwise_and)
            t2 = main.tile([P, NJ], i32)
            nc.vector.tensor_add(t2, sa, li)
            c0 = main.tile([P, NJ], i32)
            nc.vector.tensor_scalar(out=c0, in0=t2, scalar1=V - 1, scalar2=None,
                                    op0=A.bitwise_and)
            cc = main.tile([P, NJ], i32)
            nc.vector.tensor_tensor(out=cc, in0=c0, in1=zi, op=A.bitwise_or)
            cf = main.tile([P, NJ], f32)
            nc.vector.tensor_copy(cf, cc)

            # ---- z1 into out_sb + store per 2-slot chunk (SP/ACT rings) ----
            for ch in range(4):
                js = ch * 2
                nc.vector.tensor_tensor(
                    out=out_sb[:, js:js + 2, V:2 * V],
                    in0=bcast_mid(iota_f, 2),
                    in1=bcast_last(cf[:, js:js + 2], V),
                    op=A.is_equal,
                )
                eng = nc.sync if ch % 2 == 0 else nc.scalar
                eng.dma_start(out=out_r[:, js:js + 2, :], in_=out_sb[:, js:js + 2, :])

    nc.finalize()
    return nc


# Test-harness hooks: extra kwargs for run_bass_kernel_spmd (e.g. trace=True)
# and the last BassKernelResults for profiling. Unused when graded.
RUN_KWARGS: dict = {}
LAST_RESULTS = None


def kernel(**inputs) -> np.ndarray:
    global LAST_RESULTS
    from concourse.bass_utils import run_bass_kernel_spmd

    x = np.ascontiguousarray(np.asarray(inputs["inputs"], dtype=np.float32))
    W1 = np.asarray(inputs["W1"], dtype=np.float32)
    b1 = np.asarray(inputs["b1"], dtype=np.float32)
    W2 = np.asarray(inputs["W2"], dtype=np.float32)
    b2 = np.asarray(inputs["b2"], dtype=np.float32)

    # pure layout marshalling into one per-partition weight buffer
    w1t = W1.T.reshape(KH, P, V).transpose(1, 0, 2)      # [P, KH, V]
    wb = np.zeros((P, WB_COLS), np.float32)
    wb[:, W1K0_OFF:W1K0_OFF + V] = w1t[:, 0, :]
    wb[:, B1_OFF:B1_OFF + KH] = b1.reshape(KH, P).T
    wb[:, B2_OFF:B2_OFF + 2 * V] = b2.reshape(1, 2 * V)
    wb[:, W1R_OFF:W1R_OFF + 3 * V] = w1t[:, 1:, :].reshape(P, 3 * V)
    wb[:, W2_OFF:W2_OFF + KH * 2 * V] = (
        W2.reshape(KH, P, 2 * V).transpose(1, 0, 2).reshape(P, KH * 2 * V)
    )

    B = x.shape[0]
    rows = B // N_CORES
    nc = build_bass(rows)

    shards = np.split(x, N_CORES, axis=0)
    in_maps = [{"x": s, "wb": wb} for s in shards]
    res = run_bass_kernel_spmd(nc, in_maps, list(range(N_CORES)), **RUN_KWARGS)
    LAST_RESULTS = res
    return np.concatenate([r["out"] for r in res.results], axis=0)
